# revision 1
# baseline (speedup 1.0000x reference)
"""Trainium2 Bass kernel for nn_MinibatchDiscrimination (v2c, fp16 fast path).

Reference math (f32):
    M = einsum('bi,ijk->bjk', x, T)                     # [512, 64, 16]
    L1[i,j,o] = sum_k |M[i,o,k] - M[j,o,k]|             # [512, 512, 64]
    c = exp(-L1) * (1 - eye)                            # mask self-pairs
    o_b = 0.5 * c.mean(axis=1)                          # [512, 64]
    out = concat([x, o_b], axis=1)                      # [512, 320]

Sharding: the i-index of the pairwise computation is split across 8 cores
(64 rows each). SPMD-uniform: each core receives x ROTATED by -64*c rows so
its own slab lands at pair-columns j'=0..63; only input DATA differs between
cores, never addresses.

Symmetry: c[i,j]=c[j,i]; each row il processes only the 256-wide window
j' in [il+1, il+256]. Every unordered pair {a,b} with d=(b-a) mod 512:
d in [1,255] -> covered by one row's window (A-side row-sum via exp
accum_out) plus a column-partial C for the partner; d=256 -> covered by BOTH
rows' windows A-side only (C uses window cols 1..255). Host combines.

Engine plan (vs. 131.5us baseline):
  - M staged as fp16 (MT4h) + f32 per-row scalar columns (MT4C).
  - |d| = 2*relu(d) - d. The relu is ONE walrus-valid builtin
    tensor_scalar(subtract, max) per 128-partition chunk: DVE runs fp16 at
    4x_2p (0.25 cyc/elem) -> 127ns/chunk, chunks 0-5 of each row. The linear
    -d term telescopes: sum_k d_k = S[j]-S[il] over the relu-handled k; -S[j]
    enters the PSUM via ONE f32r matmul per row-PAIR (-I128 against a
    dual-row shifted S^T, ST2), +(-S[il]) rides the Exp bias (per-partition
    scalar from SB2, a strided negated copy of ST2). ST2 itself comes from
    x @ Tsum (Tsum host-precomputed) -> ready right after the input DMA.
  - chunk 6 on ACT (Abs activation, true |d|, fp8e4 out), chunk 7 on Pool
    (relu tensor_scalar, fp8e4 out); each row contracts both in ONE fp8e4
    DoubleRow matmul at 0.5 cyc/col. DoubleRow cannot target PSUM partition
    base 64, so both halves use 128-wide zero-padded indicators (dst 0,
    the unused half receives +0).
  - k-contraction of DVE chunks: 6 fp16 indicator matmuls/row (weight 2.0).
  - rows processed in PAIRS sharing one PSUM tile [128,256] (even row ->
    partitions 0:64, odd -> 64:128): ONE Exp activation per pair with
    accum_out -> A2[:, pair] (row sums for both rows at once) and bias
    -S[il] per partition.
  - column partials: ONE Pool add per pair into a dual-row C tile Cp
    [128, 320]; the odd-row half is stored shifted by -1 column so both
    halves share one instruction; host unshifts.
"""

import numpy as np
from contextlib import ExitStack

import concourse.bass as bass
import concourse.tile as tile
from concourse import bacc, mybir
from concourse.bass_utils import run_bass_kernel_spmd

F32 = mybir.dt.float32
F32R = mybir.dt.float32r
F16 = mybir.dt.float16
FP8 = mybir.dt.float8e4

B = 512          # batch
INF = 256        # in_features
OUTF = 64        # out_features
KD = 16          # kernel dims
N_CORES = 8
SLAB = B // N_CORES          # 64 rows of i per core
NU = KD // 2                 # 8 (s,o)-chunks (pairs of k)
W = 256                      # symmetric window width
MTW = 320                    # produced M^T width (max col ever read: 319)
OSCALE = 0.5 / B             # exact power of two (2^-10)
N_DVE = 6                    # chunks 0..5 -> DVE relu; 6 -> ACT abs; 7 -> Pool relu
U_ACT = 6                    # the abs chunk (excluded from S)

SUB = mybir.AluOpType.subtract
MAX = mybir.AluOpType.max
MULT = mybir.AluOpType.mult
DR = mybir.MatmulPerfMode.DoubleRow


def _build_nc(n_rows=SLAB):
    nc = bacc.Bacc("TRN2", target_bir_lowering=False, debug=False)

    # HWDGE setup costs ~625ns PER dma_start and is serialized, so inputs are
    # packed by dtype into as few DMAs as possible.
    # fp16 pack 1: per ic half: [xT(320) | Tsum(64)] then ind16(64):
    #   cols 0:320 xT_ic0, 320:384 Ts_ic0, 384:704 xT_ic1, 704:768 Ts_ic1,
    #   768:832 ind16 (2.0-weight (s,o)->o indicator; only rows of ic0 used)
    xts_d = nc.dram_tensor("xts", [128, 832], F16, kind="ExternalInput").ap()
    # fp8 pack: T host-permuted per u-chunk: col u*256 + ic*128 + s*64 + o
    # = T[ic*128+i, o, 2u+s]. fp8e4 feeds DoubleRow production matmuls (both
    # ic halves contracted in one 0.5 cyc/col pass).
    t_d = nc.dram_tensor("Tp8", [128, 2048], FP8, kind="ExternalInput").ap()
    # x^T in fp8, both ic halves as DoubleRow slots
    x8_d = nc.dram_tensor("x8", [128, 640], FP8, kind="ExternalInput").ap()
    # fp8 DoubleRow indicators, 4 slots (lo0, lo1, hi0, hi1): lo uses cols
    # 0:64 (even rows), hi cols 64:128 (odd rows); slot-pair entry 0 weight
    # 1.0 (abs chunk), entry 1 weight 2.0 (relu chunk)
    i8_d = nc.dram_tensor("ind8", [128, 512], FP8, kind="ExternalInput").ap()
    # -I128, f32r: lhs of the pair -S[j] matmul
    negi_d = nc.dram_tensor("negI", [128, 128], F32R, kind="ExternalInput").ap()
    # output: [Cp(320) | A2(32)]
    ac_d = nc.dram_tensor("outac", [128, MTW + SLAB // 2], F32,
                          kind="ExternalOutput").ap()

    with tile.TileContext(nc) as tc, ExitStack() as ctx:
        consts = ctx.enter_context(tc.tile_pool(name="consts", bufs=1))
        work = ctx.enter_context(tc.tile_pool(name="work", bufs=1))
        dpool = ctx.enter_context(tc.tile_pool(name="dpool", bufs=36))
        d8pool = ctx.enter_context(tc.tile_pool(name="d8pool", bufs=8))
        epool = ctx.enter_context(tc.tile_pool(name="epool", bufs=8))
        ps_l1 = ctx.enter_context(tc.tile_pool(name="ps_l1", bufs=4, space="PSUM"))

        # zero weights for the full-tile group-closing matmul (see main loop);
        # memset FIRST so the PE warm-up (which reads it) starts immediately
        # and the Pool-queue DMAs below don't delay it.
        zeroF = consts.tile([128, 128], F32, tag="zeroF", name="zeroF")
        nc.gpsimd.memset(zeroF, 0.0)

        # ---- inputs: big fp16 packs on the HWDGE queue, small constants on
        # the Pool engine's software DGE so their setup does not serialize
        # behind the big transfers ----
        tp8 = consts.tile([128, 8, 2, 128], FP8, tag="tp8", name="tp8")
        nc.sync.dma_start(tp8, t_d)
        x8 = consts.tile([128, 2, 320], FP8, tag="x8", name="x8")
        nc.sync.dma_start(x8, x8_d)
        xts = consts.tile([128, 832], F16, tag="xts", name="xts")
        nc.sync.dma_start(xts, xts_d)
        ind8x = consts.tile([128, 4, 128], FP8, tag="ind8x", name="ind8x")
        nc.gpsimd.dma_start(ind8x, i8_d)
        negI = consts.tile([128, 128], F32R, tag="negI", name="negI")
        nc.sync.dma_start(negI, negi_d)
        ind16 = xts[:, 768:832]
        ind8lo = ind8x[:, 0:2, :]
        ind8hi = ind8x[:, 2:4, :]

        # M^T, fp16: MT4h[:, u*MTW + j][p=(s,o)] = M[j, o, 2u+s]
        MT4h = consts.tile([128, NU * MTW], F16, tag="mt4h", name="MT4h")
        # f32 scalar columns: MT4C[:, u*SLAB + il] = M[il, o, 2u+s]
        MT4C = consts.tile([128, NU * SLAB], F32, tag="mt4c", name="MT4C")
        # negated u=5 scalar columns (Relu-on-ACT bias needs -M[il])
        MT4Cn = consts.tile([128, SLAB], F32, tag="mt4cn", name="MT4Cn")
        # dual-row shifted S^T over relu chunks (f32r):
        #   ST2[o, j]    = S[j, o] = sum_{k in relu} M[j, o, k]
        #   ST2[64+o, j] = S[j+1, o]
        ST2 = consts.tile([128, MTW], F32R, tag="st2", name="ST2")
        # -S[il] bias columns: SB2[o + 64*par, p] = -S[2p+par, o] = -ST2[:, 2p]
        SB2 = consts.tile([128, SLAB // 2], F32, tag="sb2", name="SB2")
        # combined output tile: [Cp(320) | A2(32)]
        AC = work.tile([128, MTW + SLAB // 2], F32, tag="AC", name="AC")
        Cp = AC[:, 0:MTW]
        A2 = AC[:, MTW:MTW + SLAB // 2]
        nc.vector.memset(Cp, 0.0)
        # (ST2[64:, 319] is never written NOR read: pair-S windows stop at 318)

        # ---- production: ST2/SB2 then MT4h/MT4C from host-staged x^T ----
        with tc.tile_pool(name="ps_prod", bufs=2, space="PSUM") as ps_prod, \
             tc.tile_pool(name="ps_st", bufs=1, space="PSUM") as ps_st:
            xT = [xts[:, 0:MTW], xts[:, 384:384 + MTW]]
            ts_sb = [xts[:, 320:384], xts[:, 704:768]]

            # S^T directly from x and Tsum: lower plain, upper left-shifted
            st_ps = ps_st.tile([128, MTW], F32, tag="stps", name="st_ps")
            st_ps2 = ps_st.tile([128, MTW], F32, tag="stps2", name="st_ps2")
            # PE pstate warm-up: the cost model runs PE at 0.65/1.2 GHz until
            # it has been continuously busy for 3us. Burn the input-DMA wait
            # on zero matmuls so production starts at full clock.
            for w in range(15):
                nc.tensor.matmul(st_ps[:, 0:64], zeroF, zeroF[:, 0:64],
                                 start=True, stop=True)
            for ic in range(2):
                nc.tensor.matmul(st_ps[0:64, :], ts_sb[ic], xT[ic],
                                 start=(ic == 0), stop=(ic == 1))
                nc.tensor.matmul(st_ps2[64:128, 0:MTW - 1], ts_sb[ic],
                                 xT[ic][:, 1:MTW],
                                 start=(ic == 0), stop=(ic == 1))
            nc.scalar.copy(ST2[0:64, :], st_ps[0:64, :])
            nc.vector.tensor_copy(ST2[64:128, 0:MTW - 1],
                                  st_ps2[64:128, 0:MTW - 1])
            # SB2 = -ST2[:, even cols]
            nc.vector.tensor_scalar(SB2, ST2[:, 0:SLAB:2], -1.0, None, MULT)

            for u in range(NU):
                ps = ps_prod.tile([128, MTW], F32, tag="pst", name=f"ps_mt{u}")
                nc.tensor.matmul(ps, tp8[:, u, :, :], x8,
                                 start=True, stop=True, perf_mode=DR)
                # fp16 main copy (GPSIMD cannot read PSUM -> ACT/DVE only),
                # then f32 scalar columns from the fp16 copy (cheap SBUF 2x).
                dst = MT4h[:, u * MTW:(u + 1) * MTW]
                if u % 2 == 0:
                    nc.scalar.copy(dst, ps)
                else:
                    nc.vector.tensor_copy(dst, ps)
                # SBUF->SBUF split Pool/DVE: keeps both startup queues short
                cdst = MT4C[:, u * SLAB:(u + 1) * SLAB]
                csrc = MT4h[:, u * MTW:u * MTW + SLAB]
                if u % 2 == 0:
                    nc.gpsimd.tensor_copy(cdst, csrc)
                else:
                    nc.vector.tensor_copy(cdst, csrc)
            nc.gpsimd.tensor_scalar(MT4Cn, MT4h[:, 5 * MTW:5 * MTW + SLAB],
                                    -1.0, None, MULT)

        # ---- main loop over row pairs, software-pipelined ----
        # ACT/Pool execute in program order: a pair's Exp (which waits on
        # that pair's full matmul chain) must not sit between consecutive
        # pairs' absdiff work or it serializes the steady state. Emit each
        # pair's Exp/C-add LAG pairs behind its D-production.
        LAG = 3
        l1_tiles = {}

        def emit_front(p):
            a = 2 * p
            L1 = ps_l1.tile([128, W], F32, tag="L1", name=f"L1_{p}")
            l1_tiles[p] = L1
            # -S[j] for both rows in one f32r matmul (starts the psum tile)
            nc.tensor.matmul(L1, negI, ST2[:, a + 1: a + 1 + W],
                             start=True, stop=False)
            for half in range(2):
                il = a + half
                q = 64 * half
                D8 = d8pool.tile([128, 2, W], FP8, tag="D8", name=f"D8_{il}")
                for u in range(NU):
                    src = MT4h[:, u * MTW + il + 1: u * MTW + il + 1 + W]
                    col = MT4C[:, u * SLAB + il: u * SLAB + il + 1]
                    if u < N_DVE:
                        D = dpool.tile([128, W], F16, tag="D", name=f"D_{il}_{u}")
                        if u == 5 and il % 5 == 0:
                            # relu(src - col) = Relu(+src + (-col)) on ACT,
                            # easing the DVE steady-state cap
                            nc.scalar.activation(
                                D, src, mybir.ActivationFunctionType.Relu,
                                bias=MT4Cn[:, il:il + 1], scale=1.0)
                        else:
                            nc.vector.tensor_scalar(D, src, col, 0.0, SUB, MAX)
                        nc.tensor.matmul(L1[q:q + 64, :], ind16, D,
                                         start=False, stop=False)
                    elif u == U_ACT:
                        # |col - in| = Abs(-in + bias)
                        nc.scalar.activation(
                            D8[:, 0, :], src, mybir.ActivationFunctionType.Abs,
                            bias=col, scale=-1.0)
                    else:
                        nc.gpsimd.tensor_scalar(D8[:, 1, :], src, col, 0.0,
                                                SUB, MAX)
                # DoubleRow contracts both fp8 chunks in one 0.5 cyc/col
                # matmul; the 128-wide indicator has zeros on the other
                # half's columns (dst base must be 0 in DR mode).
                nc.tensor.matmul(L1, ind8lo if half == 0 else ind8hi, D8,
                                 start=False, stop=False, perf_mode=DR)
            # the sim's psum zero-region model cannot express stops on
            # interleaved 0:64/64:128 partition groups: close the whole
            # accumulation group with one zero-weight 128-partition matmul
            # (adds 0.0, ~16 cycles).
            nc.tensor.matmul(L1[:, 0:4], zeroF, MT4C[:, 0:4],
                             start=False, stop=True)

        def emit_back(p):
            a = 2 * p
            L1 = l1_tiles.pop(p)
            E2 = epool.tile([128, W], F32, tag="E2", name=f"E2_{p}")
            nc.scalar.activation(
                E2, L1, mybir.ActivationFunctionType.Exp,
                bias=SB2[:, p:p + 1], scale=-1.0, accum_out=A2[:, p:p + 1],
            )
            # column partials, both halves in one op:
            #   even row a:      C[o, a+1+t]                  += E2[o, t]  t<255
            #   odd  row b=a+1:  C[o, b+1+t] = C[o, (a+1+t)+1] += E2[64+o, t]
            # -> odd half stored shifted by -1 col in Cp[64:], host unshifts.
            nc.gpsimd.tensor_add(
                Cp[:, a + 1: a + 1 + (W - 1)],
                Cp[:, a + 1: a + 1 + (W - 1)],
                E2[:, 0:W - 1],
            )

        npairs = n_rows // 2
        for p in range(npairs):
            emit_front(p)
            if p >= LAG:
                emit_back(p - LAG)
        for p in range(npairs - LAG, npairs):
            emit_back(p)

        nc.sync.dma_start(ac_d, AC)

    nc.compile()
    return nc


_NC = None


def _get_nc():
    global _NC
    if _NC is None:
        _NC = _build_nc()
    return _NC


def _host_inputs(x, T):
    f8 = mybir.dt.np(FP8)
    ind = np.zeros((128, OUTF), np.float32)
    ind[np.arange(128), np.arange(128) % OUTF] = 1.0
    # DoubleRow indicators, 4 slots (lo0, lo1, hi0, hi1): slot-pair entry 0
    # weight 1.0 (abs), entry 1 weight 2.0 (relu); lo cols 0:64, hi 64:128.
    i8 = np.zeros((128, 4, 128), np.float32)
    i8[:, 0, 0:64] = ind
    i8[:, 1, 0:64] = 2.0 * ind
    i8[:, 2, 64:128] = ind
    i8[:, 3, 64:128] = 2.0 * ind
    negI = (-np.eye(128)).astype(np.float32)
    # [i, o, (u s)] -> [i, (u s o)], both ic halves side by side
    # Tp8[i, u*256 + ic*128 + s*64 + o] = T[ic*128+i, o, 2u+s], fp8e4
    Tp = T.reshape(INF, OUTF, NU, 2).transpose(0, 2, 3, 1)  # [i, u, s, o]
    Tp8 = np.ascontiguousarray(
        Tp.reshape(2, 128, NU, 2 * OUTF).transpose(1, 2, 0, 3).reshape(128, 2048)
    ).astype(f8)
    # Tsum over relu-handled k (all but 2*U_ACT, 2*U_ACT+1)
    kmask = np.ones(KD, bool)
    kmask[2 * U_ACT:2 * U_ACT + 2] = False
    Ts = T[:, :, kmask].sum(axis=2).astype(np.float16)
    in_maps = []
    for c in range(N_CORES):
        xr = np.roll(x, -c * SLAB, axis=0)
        xrT = np.ascontiguousarray(xr[0:MTW, :].T).astype(np.float16)
        xts = np.concatenate([
            xrT[0:128, :], Ts[0:128, :],      # 0:320 | 320:384
            xrT[128:256, :], Ts[128:256, :],  # 384:704 | 704:768
            (2.0 * ind).astype(np.float16),   # 768:832
        ], axis=1).astype(np.float16)
        x8 = np.ascontiguousarray(
            xrT.reshape(2, 128, MTW).transpose(1, 0, 2).reshape(128, 640)
        ).astype(f8)
        in_maps.append({
            "xts": xts, "Tp8": Tp8, "x8": x8,
            "ind8": i8.reshape(128, 512).astype(f8),
            "negI": negI,
        })
    return in_maps


def _assemble(x, results):
    """Combine per-core row-sums and column-partials into the full output."""
    At = np.zeros((B, OUTF), np.float64)
    jj = np.arange(MTW)
    for c in range(N_CORES):
        ac = np.asarray(results[c]["outac"])  # [128, 352]
        cp, a2 = ac[:, 0:MTW], ac[:, MTW:]
        rows = c * SLAB + np.arange(0, SLAB, 2)
        At[rows, :] += a2[0:64, :].T         # even rows
        At[rows + 1, :] += a2[64:128, :].T   # odd rows
        np.add.at(At, (jj + c * SLAB) % B, cp[0:64, :].T.astype(np.float64))
        np.add.at(At, (jj + 1 + c * SLAB) % B, cp[64:128, :].T.astype(np.float64))
    o_b = (At * OSCALE).astype(np.float32)
    return np.concatenate([x, o_b], axis=1)


def _run(x, T, trace=False):
    x = np.ascontiguousarray(np.asarray(x, dtype=np.float32))
    T = np.ascontiguousarray(np.asarray(T, dtype=np.float32))
    assert x.shape == (B, INF) and T.shape == (INF, OUTF, KD)
    nc = _get_nc()
    in_maps = _host_inputs(x, T)
    res = run_bass_kernel_spmd(nc, in_maps, list(range(N_CORES)), trace=trace)
    return _assemble(x, res.results), res


def kernel(x, T):
    out, _ = _run(x, T, trace=False)
    return out


def kernel_profiled(x, T):
    out, res = _run(x, T, trace=True)
    return out, res

